# revision 3
# baseline (speedup 1.0000x reference)
"""HeteroRGCN (2-layer, 4 relations) distributed across 8 NeuronCores.

Sharding strategy (per spec sharding_hint):
  - Transaction (t) nodes: contiguous 8-way shard (62500 rows/core); their
    incident edges are partitioned with them (c2t/m2t edges live on the core
    owning the *dst* t-node; t2c/t2m edges on the core owning the *src*
    t-node), so all message gathers are core-local.
  - Tiny per-etype weight matrices: replicated.
  - Client/merchant tables are small: wh_c / wh_m are computed replicated;
    the t->c / t->m segment-mean accumulators are computed as per-core
    partials and combined with an all-reduce (psum) across the 8 cores
    (the "halo exchange" of boundary aggregates).
  - Segment-mean is folded into a per-edge weight (1/deg[dst], 0 for pad
    edges) precomputed on host from the integer edge lists; biases pass
    through the mean and are added post-aggregation gated by (deg>0),
    matching DGL zero-in-degree semantics.
  - The neuron compiler crashes when a gather and a scatter-add land in the
    same XLA module, so each layer is split into a gather stage (matmuls +
    edge gathers) and a scatter stage (segment sums + all-reduce + bias /
    activation); intermediates stay device-resident between stages.
"""
import numpy as np
import jax
import jax.numpy as jnp

NT, NC_, NM = 500_000, 100_000, 20_000
E = 500_000
IN, EMB, HID, OUT = 128, 64, 64, 2
NCORES = 8
TS = NT // NCORES   # 62500 t-rows per core

_DEVS = jax.devices()[:NCORES]


def _stage_gather(h_t, h_c, h_m, W,
                  c2t_s, c2t_w, m2t_s, m2t_w, t2c_s, t2c_w, t2m_s, t2m_w):
    wh_c = h_c @ W["c2t"]
    wh_m = h_m @ W["m2t"]
    wh_tA = h_t @ W["t2c"]
    wh_tB = h_t @ W["t2m"]
    m_c2t = wh_c[c2t_s] * c2t_w[:, None]
    m_m2t = wh_m[m2t_s] * m2t_w[:, None]
    m_t2c = wh_tA[t2c_s] * t2c_w[:, None]
    m_t2m = wh_tB[t2m_s] * t2m_w[:, None]
    return m_c2t, m_m2t, m_t2c, m_t2m


def _stage_scatter(m_c2t, m_m2t, m_t2c, m_t2m,
                   c2t_d, m2t_d, t2c_d, t2m_d,
                   g_t_c2t, g_t_m2t, g_c, g_m, b, relu):
    a_t = (jax.ops.segment_sum(m_c2t, c2t_d, num_segments=TS)
           + jax.ops.segment_sum(m_m2t, m2t_d, num_segments=TS)
           + g_t_c2t * b["c2t"] + g_t_m2t * b["m2t"])
    p_c = jax.ops.segment_sum(m_t2c, t2c_d, num_segments=NC_)
    p_m = jax.ops.segment_sum(m_t2m, t2m_d, num_segments=NM)
    a_c = jax.lax.psum(p_c, "x") + g_c * b["t2c"]
    a_m = jax.lax.psum(p_m, "x") + g_m * b["t2m"]
    if relu:
        a_t = jax.nn.leaky_relu(a_t)
        a_c = jax.nn.leaky_relu(a_c)
        a_m = jax.nn.leaky_relu(a_m)
    return a_t, a_c, a_m


_F_GATHER = jax.pmap(_stage_gather, axis_name="x", devices=_DEVS)
_F_SCATTER0 = jax.pmap(lambda *a: _stage_scatter(*a, relu=True),
                       axis_name="x", devices=_DEVS)
_F_SCATTER1 = jax.pmap(lambda *a: _stage_scatter(*a, relu=False),
                       axis_name="x", devices=_DEVS)
_F_FINAL = jax.pmap(lambda h, Wf, bf: h @ Wf + bf, axis_name="x", devices=_DEVS)


def _bucket_edges(src, dst, key, nbuck, bsize):
    """Partition edges by key//bsize into nbuck buckets; pad to common length.
    Per-edge weight is 1/deg[dst] (0 on pads) so weighted segment-sum == mean."""
    src = np.asarray(src, np.int64)
    dst = np.asarray(dst, np.int64)
    deg = np.bincount(dst)
    b = np.asarray(key, np.int64) // bsize
    order = np.argsort(b, kind="stable")
    sb, db, bb = src[order], dst[order], b[order]
    counts = np.bincount(bb, minlength=nbuck)
    off = np.zeros(nbuck + 1, np.int64)
    np.cumsum(counts, out=off[1:])
    L = max(int(counts.max()), 1)
    S = np.zeros((nbuck, L), np.int32)
    D = np.zeros((nbuck, L), np.int32)
    W = np.zeros((nbuck, L), np.float32)
    for k in range(nbuck):
        s, e = off[k], off[k + 1]
        n = e - s
        S[k, :n] = sb[s:e]
        D[k, :n] = db[s:e]
        W[k, :n] = 1.0 / np.maximum(deg[db[s:e]], 1)
    return S, D, W


def kernel(**inputs) -> np.ndarray:
    feat = np.asarray(inputs["features"], np.float32)
    embc = np.asarray(inputs["emb_client"], np.float32)
    embm = np.asarray(inputs["emb_merchant"], np.float32)

    idx = {k: np.asarray(inputs[k], np.int64)
           for k in ["src_c2t", "dst_c2t", "src_m2t", "dst_m2t",
                     "src_t2c", "dst_t2c", "src_t2m", "dst_t2m"]}

    # ---- host-side graph partitioning (integer-only index prep) ----
    c2t_S, c2t_D, c2t_W = _bucket_edges(idx["src_c2t"], idx["dst_c2t"], idx["dst_c2t"], NCORES, TS)
    c2t_D = (c2t_D % TS).astype(np.int32)
    m2t_S, m2t_D, m2t_W = _bucket_edges(idx["src_m2t"], idx["dst_m2t"], idx["dst_m2t"], NCORES, TS)
    m2t_D = (m2t_D % TS).astype(np.int32)
    t2c_S, t2c_D, t2c_W = _bucket_edges(idx["src_t2c"], idx["dst_t2c"], idx["src_t2c"], NCORES, TS)
    t2c_S = (t2c_S % TS).astype(np.int32)
    t2m_S, t2m_D, t2m_W = _bucket_edges(idx["src_t2m"], idx["dst_t2m"], idx["src_t2m"], NCORES, TS)
    t2m_S = (t2m_S % TS).astype(np.int32)

    # bias gates: 1.0 where in-degree > 0 (per relation, per dst node)
    deg_t_c2t = np.bincount(idx["dst_c2t"], minlength=NT).reshape(NCORES, TS, 1)
    deg_t_m2t = np.bincount(idx["dst_m2t"], minlength=NT).reshape(NCORES, TS, 1)
    deg_c = np.bincount(idx["dst_t2c"], minlength=NC_).reshape(NC_, 1)
    deg_m = np.bincount(idx["dst_t2m"], minlength=NM).reshape(NM, 1)
    g_t_c2t = (deg_t_c2t > 0).astype(np.float32)
    g_t_m2t = (deg_t_m2t > 0).astype(np.float32)
    g_c = np.broadcast_to((deg_c > 0).astype(np.float32), (NCORES, NC_, 1)).copy()
    g_m = np.broadcast_to((deg_m > 0).astype(np.float32), (NCORES, NM, 1)).copy()

    def rep(x):
        x = np.asarray(x, np.float32)
        return np.broadcast_to(x, (NCORES,) + x.shape).copy()

    W0 = {e: rep(inputs[f"W0_{e}"]) for e in ["c2t", "m2t", "t2c", "t2m"]}
    b0 = {e: rep(inputs[f"b0_{e}"]) for e in ["c2t", "m2t", "t2c", "t2m"]}
    W1 = {e: rep(inputs[f"W1_{e}"]) for e in ["c2t", "m2t", "t2c", "t2m"]}
    b1 = {e: rep(inputs[f"b1_{e}"]) for e in ["c2t", "m2t", "t2c", "t2m"]}

    h_t = feat.reshape(NCORES, TS, IN)
    h_c, h_m = rep(embc), rep(embm)

    for layer, (W, b, scat) in enumerate([(W0, b0, _F_SCATTER0),
                                          (W1, b1, _F_SCATTER1)]):
        mc, mm, mtc, mtm = _F_GATHER(h_t, h_c, h_m, W,
                                     c2t_S, c2t_W, m2t_S, m2t_W,
                                     t2c_S, t2c_W, t2m_S, t2m_W)
        h_t, h_c, h_m = scat(mc, mm, mtc, mtm,
                             c2t_D, m2t_D, t2c_D, t2m_D,
                             g_t_c2t, g_t_m2t, g_c, g_m, b)

    out = _F_FINAL(h_t, rep(inputs["Wf"]), rep(inputs["bf"]))
    out = np.asarray(out).reshape(NT, OUT)
    return out.astype(np.float32)


# revision 4
# speedup vs baseline: 1.7434x; 1.7434x over previous
"""HeteroRGCN (2-layer, 4 relations) distributed across 8 NeuronCores.

Sharding strategy (per spec sharding_hint):
  - Transaction (t) nodes: contiguous 8-way shard (62500 rows/core); their
    incident edges are partitioned with them (c2t/m2t edges live on the core
    owning the *dst* t-node; t2c/t2m edges on the core owning the *src*
    t-node), so all message gathers are core-local.
  - Tiny per-etype weight matrices: replicated.
  - Client/merchant tables are small: wh_c / wh_m are computed replicated;
    the t->c / t->m segment-mean accumulators are computed as per-core
    partials and combined with an all-reduce (psum) across the 8 cores
    (the "halo exchange" of boundary aggregates).
  - Segment-mean is folded into a per-edge weight (1/deg[dst], 0 for pad
    edges) precomputed on host from the integer edge lists; biases pass
    through the mean and are added post-aggregation gated by (deg>0),
    matching DGL zero-in-degree semantics.
  - The neuron compiler crashes when a gather and a scatter-add land in the
    same XLA module, so each layer is split into a gather stage (matmuls +
    edge gathers) and a scatter stage (segment sums + all-reduce + bias /
    activation); intermediates stay device-resident between stages.
"""
import numpy as np
import jax
import jax.numpy as jnp

NT, NC_, NM = 500_000, 100_000, 20_000
E = 500_000
IN, EMB, HID, OUT = 128, 64, 64, 2
NCORES = 8
TS = NT // NCORES   # 62500 t-rows per core

_DEVS = jax.devices()[:NCORES]


def _stage_gather(h_t, h_c, h_m, W,
                  c2t_s, c2t_w, m2t_s, m2t_w, t2c_s, t2c_w, t2m_s, t2m_w):
    wh_c = h_c @ W["c2t"]
    wh_m = h_m @ W["m2t"]
    if h_c.shape[0] != NC_:   # layer 0: emb tables arrive 8-way sharded
        wh_c = jax.lax.all_gather(wh_c, "x", tiled=True)
        wh_m = jax.lax.all_gather(wh_m, "x", tiled=True)
    wh_tA = h_t @ W["t2c"]
    wh_tB = h_t @ W["t2m"]
    m_c2t = wh_c[c2t_s] * c2t_w[:, None]
    m_m2t = wh_m[m2t_s] * m2t_w[:, None]
    m_t2c = wh_tA[t2c_s] * t2c_w[:, None]
    m_t2m = wh_tB[t2m_s] * t2m_w[:, None]
    return m_c2t, m_m2t, m_t2c, m_t2m


def _stage_scatter(m_c2t, m_m2t, m_t2c, m_t2m,
                   c2t_d, m2t_d, t2c_d, t2m_d,
                   g_t_c2t, g_t_m2t, g_c, g_m, b, relu):
    a_t = (jax.ops.segment_sum(m_c2t, c2t_d, num_segments=TS)
           + jax.ops.segment_sum(m_m2t, m2t_d, num_segments=TS)
           + g_t_c2t * b["c2t"] + g_t_m2t * b["m2t"])
    p_c = jax.ops.segment_sum(m_t2c, t2c_d, num_segments=NC_)
    p_m = jax.ops.segment_sum(m_t2m, t2m_d, num_segments=NM)
    a_c = jax.lax.psum(p_c, "x") + g_c * b["t2c"]
    a_m = jax.lax.psum(p_m, "x") + g_m * b["t2m"]
    if relu:
        a_t = jax.nn.leaky_relu(a_t)
        a_c = jax.nn.leaky_relu(a_c)
        a_m = jax.nn.leaky_relu(a_m)
    return a_t, a_c, a_m


def _stage_scatter_final(m_c2t, m_m2t, m_t2c, m_t2m,
                         c2t_d, m2t_d, g_t_c2t, g_t_m2t, b, Wf, bf):
    a_t = (jax.ops.segment_sum(m_c2t, c2t_d, num_segments=TS)
           + jax.ops.segment_sum(m_m2t, m2t_d, num_segments=TS)
           + g_t_c2t * b["c2t"] + g_t_m2t * b["m2t"])
    return a_t @ Wf + bf


_F_GATHER = jax.pmap(_stage_gather, axis_name="x", devices=_DEVS)
_F_SCATTER0 = jax.pmap(lambda *a: _stage_scatter(*a, relu=True),
                       axis_name="x", devices=_DEVS)
_F_SCATTER_FINAL = jax.pmap(_stage_scatter_final, axis_name="x", devices=_DEVS)


def _bucket_edges(src, dst, key, nbuck, bsize):
    """Partition edges by key//bsize into nbuck buckets; pad to common length.
    Per-edge weight is 1/deg[dst] (0 on pads) so weighted segment-sum == mean."""
    src = np.asarray(src, np.int64)
    dst = np.asarray(dst, np.int64)
    deg = np.bincount(dst)
    b = np.asarray(key, np.int64) // bsize
    order = np.argsort(b, kind="stable")
    sb, db, bb = src[order], dst[order], b[order]
    counts = np.bincount(bb, minlength=nbuck)
    off = np.zeros(nbuck + 1, np.int64)
    np.cumsum(counts, out=off[1:])
    L = max(int(counts.max()), 1)
    S = np.zeros((nbuck, L), np.int32)
    D = np.zeros((nbuck, L), np.int32)
    W = np.zeros((nbuck, L), np.float32)
    for k in range(nbuck):
        s, e = off[k], off[k + 1]
        n = e - s
        S[k, :n] = sb[s:e]
        D[k, :n] = db[s:e]
        W[k, :n] = 1.0 / np.maximum(deg[db[s:e]], 1)
    return S, D, W


def kernel(**inputs) -> np.ndarray:
    feat = np.asarray(inputs["features"], np.float32)
    embc = np.asarray(inputs["emb_client"], np.float32)
    embm = np.asarray(inputs["emb_merchant"], np.float32)

    idx = {k: np.asarray(inputs[k], np.int64)
           for k in ["src_c2t", "dst_c2t", "src_m2t", "dst_m2t",
                     "src_t2c", "dst_t2c", "src_t2m", "dst_t2m"]}

    # ---- host-side graph partitioning (integer-only index prep) ----
    c2t_S, c2t_D, c2t_W = _bucket_edges(idx["src_c2t"], idx["dst_c2t"], idx["dst_c2t"], NCORES, TS)
    c2t_D = (c2t_D % TS).astype(np.int32)
    m2t_S, m2t_D, m2t_W = _bucket_edges(idx["src_m2t"], idx["dst_m2t"], idx["dst_m2t"], NCORES, TS)
    m2t_D = (m2t_D % TS).astype(np.int32)
    t2c_S, t2c_D, t2c_W = _bucket_edges(idx["src_t2c"], idx["dst_t2c"], idx["src_t2c"], NCORES, TS)
    t2c_S = (t2c_S % TS).astype(np.int32)
    t2m_S, t2m_D, t2m_W = _bucket_edges(idx["src_t2m"], idx["dst_t2m"], idx["src_t2m"], NCORES, TS)
    t2m_S = (t2m_S % TS).astype(np.int32)

    # bias gates: 1.0 where in-degree > 0 (per relation, per dst node)
    deg_t_c2t = np.bincount(idx["dst_c2t"], minlength=NT).reshape(NCORES, TS, 1)
    deg_t_m2t = np.bincount(idx["dst_m2t"], minlength=NT).reshape(NCORES, TS, 1)
    deg_c = np.bincount(idx["dst_t2c"], minlength=NC_).reshape(NC_, 1)
    deg_m = np.bincount(idx["dst_t2m"], minlength=NM).reshape(NM, 1)
    g_t_c2t = (deg_t_c2t > 0).astype(np.float32)
    g_t_m2t = (deg_t_m2t > 0).astype(np.float32)
    g_c = np.broadcast_to((deg_c > 0).astype(np.float32), (NCORES, NC_, 1)).copy()
    g_m = np.broadcast_to((deg_m > 0).astype(np.float32), (NCORES, NM, 1)).copy()

    def rep(x):
        x = np.asarray(x, np.float32)
        return np.broadcast_to(x, (NCORES,) + x.shape).copy()

    W0 = {e: rep(inputs[f"W0_{e}"]) for e in ["c2t", "m2t", "t2c", "t2m"]}
    b0 = {e: rep(inputs[f"b0_{e}"]) for e in ["c2t", "m2t", "t2c", "t2m"]}
    W1 = {e: rep(inputs[f"W1_{e}"]) for e in ["c2t", "m2t", "t2c", "t2m"]}
    b1 = {e: rep(inputs[f"b1_{e}"]) for e in ["c2t", "m2t", "t2c", "t2m"]}

    h_t = feat.reshape(NCORES, TS, IN)
    h_c = embc.reshape(NCORES, NC_ // NCORES, EMB)   # sharded; all_gather on device
    h_m = embm.reshape(NCORES, NM // NCORES, EMB)

    mc, mm, mtc, mtm = _F_GATHER(h_t, h_c, h_m, W0,
                                 c2t_S, c2t_W, m2t_S, m2t_W,
                                 t2c_S, t2c_W, t2m_S, t2m_W)
    h_t, h_c, h_m = _F_SCATTER0(mc, mm, mtc, mtm,
                                c2t_D, m2t_D, t2c_D, t2m_D,
                                g_t_c2t, g_t_m2t, g_c, g_m, b0)
    mc, mm, mtc, mtm = _F_GATHER(h_t, h_c, h_m, W1,
                                 c2t_S, c2t_W, m2t_S, m2t_W,
                                 t2c_S, t2c_W, t2m_S, t2m_W)
    out = _F_SCATTER_FINAL(mc, mm, mtc, mtm, c2t_D, m2t_D,
                           g_t_c2t, g_t_m2t, b1,
                           rep(inputs["Wf"]), rep(inputs["bf"]))
    out = np.asarray(out).reshape(NT, OUT)
    return out.astype(np.float32)
